# revision 32
# baseline (speedup 1.0000x reference)
"""ArcFace loss kernel for 8 Trainium2 NeuronCores.

Strategy (classification/tensor parallel over the class dim C):
  - Host prep: l2-normalize x and weight rows, quantize x_n*16 and
    w_n*16 to fp8-e4m3 and pack the weight shard TRANSPOSED in
    DoubleRow pair layout [ki, (P, s, w)] (d = P*256 + s*128 + ki) so
    each core streams matmul-ready fp8 tiles straight from DRAM.
    Shards are padded with 44 zero classes (to 12544); exp(0)=1 per
    pad column is folded into the host-side correction.
    The ENTIRE label-column (margin) path is computed on host in f64:
    cos_l, phi', the sum-exp correction hcorr_b = e^{S phi'} -
    e^{S cos_l} - PADS, and the per-row affine tail k_b (which folds
    the label-smoothing sum_logits term via wsum = sum_c w_n). Only
    hcorr is shipped to the device ([128, 2*N_BT] f32, with k folded
    into a single host scalar via the final mean).
  - Device per core: cosine*256 = xq @ wq^T via fp8 DoubleRow matmuls
    (K=256/pass, 2 passes) into PSUM [b=128, c<=2048]; ACT Exp
    (scale=S/256) emits bf16 exp tiles (pure exp, no accumulators on
    ACT -- it is the critical engine); DVE tensor_scalar on bf16
    accumulates the per-b sum of exp into per-(m,block) slots.
    A dummy AllReduce at kernel start warms the CC stream / device
    barrier off the critical path. After the last block: fold slots,
    AllReduce the [128, N_BT] sum-exp, then add hcorr, Ln, and a
    ones-matmul mean with the host constant K folded into the final
    scale -- 4 small ops after the collective.

  loss_b = ln(sum_exp_b + hcorr_b) + k_b, averaged over b.
"""

import math
import sys

import numpy as np

S = 30.0
MARGIN = 0.5
EPS = 0.1
COS_M = math.cos(MARGIN)
SIN_M = math.sin(MARGIN)
TH = math.cos(math.pi - MARGIN)
MM = math.sin(math.pi - MARGIN) * MARGIN

B, D, C = 512, 512, 100000
N_CORES = 8
C_SHARD = C // N_CORES            # 12500
C_PAD = 12544                     # +44 zero classes -> 16-aligned widths
PAD_TOTAL = float((C_PAD - C_SHARD) * N_CORES)  # 352 pad columns overall
WIDTHS = [256, 512, 1536, 2048, 2048, 2048, 2048, 2048]  # sum=12544
N_BLK = len(WIDTHS)
# Per-(block, m) engine assignment, interleaved so ACT and DVE stay busy
# concurrently through every block (a block-level split starves one side:
# PSUM is only 2 tiles deep). 2048-blocks run 1-2 Schraudolph tiles on DVE
# + the rest as ACT exp+accum; small blocks 0-2 are ACT exp + DVE accum.
SCH_TILES = {(3, 0), (4, 0), (5, 0), (6, 0), (7, 0), (4, 2), (6, 2)}
ACT_ACC_TILES = {(b, m) for b in range(3, 8) for m in range(4)} - SCH_TILES
# Schraudolph fast-exp: exp(y) ~= bitcast_f32(int32(A*y + B)). The PSUM
# value is 256*cos, so fold S/256 into A. B is the float32 magic constant.
SCH_A = 12102203.1616 * (S / 256.0)
SCH_B = 1064866805.0
SCH_ZERO = float(np.array([1064866805], np.int32).view(np.float32)[0])
N_BT = B // 128                   # 4 b-tiles
ALPHA = 16.0                      # fp8 pre-scale for x_n and w_n
EXP_SCALE = S / (ALPHA * ALPHA)
N_WARM = 8                        # PE p-state warmup matmuls

_REPO = "/opt/trn_rl_repo"
if _REPO not in sys.path:
    sys.path.insert(0, _REPO)

_CACHE = {}


def _build():
    import concourse.bass as bass
    import concourse.mybir as mybir
    from concourse.tile import TileContext

    dt = mybir.dt
    Alu = mybir.AluOpType
    Act = mybir.ActivationFunctionType
    DR = mybir.MatmulPerfMode.DoubleRow

    nc = bass.Bass(num_devices=N_CORES)

    # I/O (per-core shards fed via in_maps)
    xt = nc.declare_dram_parameter("xt", [128, 2 * 1024], dt.uint8,
                                   isOutput=False)  # fp8 x_n^T DR pairs
    wt = nc.declare_dram_parameter("wt", [C_PAD * D], dt.uint8,
                                   isOutput=False)  # fp8 block images, flat
    out = nc.declare_dram_parameter("out", [128, N_BT], dt.float32,
                                    isOutput=True)  # per-core sum-exp

    f32 = dt.float32
    bf16 = dt.bfloat16
    u8 = dt.uint8
    f8 = dt.float8e4
    i32 = dt.int32

    with TileContext(nc) as tc:
        with (
            tc.tile_pool(name="wpool", bufs=4) as wpool,
            tc.tile_pool(name="io", bufs=1) as io,
            tc.tile_pool(name="escr", bufs=4) as escr,
            tc.tile_pool(name="slots", bufs=1) as slots,
            tc.tile_pool(name="stats", bufs=16) as stats,
            tc.tile_pool(name="psum", bufs=2, space="PSUM") as pp,
            tc.tile_pool(name="dram", bufs=4, space="DRAM") as dram,
        ):
            # ---- input staging: first weight block, then x image ----
            xt8 = io.tile([128, 2 * 1024], u8, tag="xt8")
            off = 0
            W0 = WIDTHS[0]
            w_first = wpool.tile([128, 4 * 2048], u8, tag="w")
            img0 = wt[0:128 * 4 * W0].rearrange("(p w) -> p w", p=128)
            nc.sync.dma_start(out=w_first[:, 0:4 * W0], in_=img0[:, :])
            off += 128 * 4 * W0
            nc.sync.dma_start(out=xt8[:, :], in_=xt[:, :])
            xt4 = xt8[:, 0:2048].rearrange("p (P s b) -> p P s b", P=2, s=2)

            sexp_slots = slots.tile([128, N_BT * N_BLK], f32, tag="sexp")

            cc_in = slots.tile([128, N_BT], f32, tag="ccin")

            # ---- PE p-state warmup on a memset tile (overlaps input DMA)
            wrm = stats.tile([128, 1024], bf16, tag="wrm")
            nc.vector.memset(wrm[:, :], 0.0)
            for wi in range(N_WARM):
                ps_w = pp.tile([128, 2048], f32, tag="cos")
                nc.tensor.matmul(ps_w[:, 0:512], lhsT=wrm[:, 0:128],
                                 rhs=wrm[:, 0:512], start=True, stop=True)

            # ---- main streaming loop over class blocks (fp8 DoubleRow) ----
            bi2 = 0
            for W in WIDTHS:
                if bi2 == 0:
                    wtile = w_first
                else:
                    wtile = wpool.tile([128, 4 * 2048], u8, tag="w")
                    img = wt[off:off + 128 * 4 * W].rearrange("(p w) -> p w",
                                                              p=128)
                    if W <= 1536:
                        nc.sync.dma_start(out=wtile[:, 0:4 * W], in_=img[:, :])
                    else:
                        hw = 4 * W // 2
                        nc.sync.dma_start(out=wtile[:, 0:hw], in_=img[:, 0:hw])
                        nc.sync.dma_start(out=wtile[:, hw:4 * W],
                                          in_=img[:, hw:4 * W])
                    off += 128 * 4 * W
                w4 = wtile[:, 0:4 * W].rearrange("p (P s w) -> p P s w",
                                                 P=2, s=2)

                for m in range(N_BT):
                    ps = pp.tile([128, 2048], f32, tag="cos")
                    for reg in range(0, W, 512):
                        RW = min(512, W - reg)
                        for P in range(2):
                            nc.tensor.matmul(
                                ps[:, reg:reg + RW],
                                lhsT=xt4[:, P, :, m * 128:(m + 1) * 128].bitcast(f8),
                                rhs=w4[:, P, :, reg:reg + RW].bitcast(f8),
                                start=(P == 0), stop=(P == 1),
                                perf_mode=DR)
                    acc = sexp_slots[:, m * N_BLK + bi2:m * N_BLK + bi2 + 1]
                    if (bi2, m) in SCH_TILES:
                        # DVE fast-exp: affine into int32 bits, then reduce
                        # the bitcast floats. Offloads ~1/6 of the exp work
                        # from the saturated ACT engine.
                        it = escr.tile([128, 2048], i32, tag="si")
                        nc.vector.tensor_scalar(
                            out=it[:, 0:W], in0=ps[:, 0:W], scalar1=SCH_A,
                            scalar2=SCH_B, op0=Alu.mult, op1=Alu.add)
                        nc.vector.reduce_sum(acc, it[:, 0:W].bitcast(f32),
                                             axis=mybir.AxisListType.X)
                    elif (bi2, m) in ACT_ACC_TILES:
                        # ACT accumulates in-pass (+283ns READ_ACC);
                        # relieves the 1x-rate DVE accumulate path.
                        esc = escr.tile([128, 2048], bf16, tag="et")
                        nc.scalar.activation(esc[:, 0:W], ps[:, 0:W],
                                             Act.Exp, scale=EXP_SCALE,
                                             accum_out=acc)
                    else:
                        esc = escr.tile([128, 2048], bf16, tag="et")
                        nc.scalar.activation(esc[:, 0:W], ps[:, 0:W],
                                             Act.Exp, scale=EXP_SCALE)
                        nc.vector.reduce_sum(acc, esc[:, 0:W],
                                             axis=mybir.AxisListType.X)
                bi2 += 1

            # ---- fold slots -> per-core sum-exp out (host combines) ----
            for m in range(N_BT):
                nc.vector.reduce_sum(
                    cc_in[:, m:m + 1],
                    sexp_slots[:, m * N_BLK:(m + 1) * N_BLK],
                    axis=mybir.AxisListType.X)
            nc.sync.dma_start(out=out[:, :], in_=cc_in[:, :])

    _split_waits(nc, mybir)
    return nc


def _split_waits(nc, mybir):
    """Walrus codegen limits most ISA structs to ONE sync-wait command.

    Tile's scheduler freely attaches several waits to any instruction, so
    hoist surplus waits onto inserted same-engine NoOps (one wait each),
    which execute (and block the engine) immediately before the real
    instruction.
    """
    import bass_rust

    idx = 0
    for blk in nc.main_func.blocks:
        il = blk.instructions
        out = []
        changed = False
        for ins in il:
            si = ins.sync_info
            nwaits = len(si.on_wait) if si is not None else 0
            if si is not None and nwaits > 1:
                waits = list(si.on_wait)
                for w in waits[1:]:
                    noop = mybir.InstNoOp(name=f"I-wsplit-{idx}", ins=[],
                                          outs=[])
                    idx += 1
                    noop.engine = ins.engine
                    noop.sync_info = bass_rust.SyncInfo(on_wait=[w],
                                                        on_update=[])
                    out.append(noop)
                ins.sync_info = bass_rust.SyncInfo(
                    on_wait=waits[:1], on_update=list(si.on_update))
                changed = True
            out.append(ins)
        if changed:
            blk.instructions = out
    return nc


def _to_fp8(a):
    import ml_dtypes
    return np.asarray(a, dtype=ml_dtypes.float8_e4m3)


def _pack_pairs_t(mat_t, width):
    """[512, width] (d-major) -> [128, 4*width] bytes, d = P*256+s*128+ki."""
    return np.ascontiguousarray(
        mat_t.reshape(2, 2, 128, width).transpose(2, 0, 1, 3)
        .reshape(128, 4 * width))


def _prep_inputs(x, weight, label):
    x = np.ascontiguousarray(np.asarray(x, dtype=np.float32))
    weight = np.ascontiguousarray(np.asarray(weight, dtype=np.float32))
    lab = np.asarray(label).astype(np.int64)

    xn = x / np.maximum(np.sqrt((x * x).sum(-1, keepdims=True)), 1e-12)
    wnorm = np.maximum(np.sqrt((weight * weight).sum(-1, keepdims=True)),
                       1e-12)
    wn_all = weight / wnorm

    # ---- host-side label-column (margin) path, all in f64 ----
    wl_n = wn_all[lab].astype(np.float64)                   # [B, D]
    xn64 = xn.astype(np.float64)
    cos_l = np.einsum("bd,bd->b", xn64, wl_n)               # [B]
    sine = np.sqrt(np.clip(1.0 - cos_l * cos_l, 0.0, 1.0))
    phi = cos_l * COS_M - sine * SIN_M
    phip = np.where(cos_l > TH, phi, cos_l - MM)            # phi'
    wsum = wn_all.sum(axis=0, dtype=np.float64)             # [D]
    sumcos = xn64 @ wsum                                    # [B]
    # Pad columns (44 per core) sit in the last block, handled by ACT exp:
    # exp(0)=1 each.
    hcorr = np.exp(S * phip) - np.exp(S * cos_l) - PAD_TOTAL
    kvec = (-(1.0 - EPS) * S * phip
            - (EPS / C) * S * (sumcos - cos_l + phip))      # [B]

    # fp8 x image
    xq = _to_fp8(xn * ALPHA)                                # [B, D]
    xt_pairs = _pack_pairs_t(np.ascontiguousarray(xq.T), B)  # [128, 2048]
    xt_p = np.ascontiguousarray(xt_pairs.view(np.uint8))

    in_maps = []
    for i in range(N_CORES):
        shn = wn_all[i * C_SHARD:(i + 1) * C_SHARD]
        shq = np.zeros((C_PAD, D), dtype=xq.dtype)
        shq[:C_SHARD] = _to_fp8(shn * ALPHA)
        wqT = np.ascontiguousarray(shq.T)        # [D, C_PAD] fp8
        parts = []
        o = 0
        for w in WIDTHS:
            parts.append(_pack_pairs_t(wqT[:, o:o + w], w).reshape(-1))
            o += w
        wt_i = np.ascontiguousarray(
            np.concatenate(parts)).view(np.uint8)  # flat [C_PAD * D]
        in_maps.append({"xt": xt_p, "wt": wt_i})
    return in_maps, hcorr, kvec


def _ensure_ntff_hook():
    """Register the axon NTFF profile hook (missing antenv.axon_hooks shim)."""
    import types

    if "antenv.axon_hooks" in sys.modules:
        return True
    try:
        mod = types.ModuleType("antenv.axon_hooks")
        state = {"hook": None}
        mod.set_axon_ntff_profile_hook = lambda h: state.__setitem__("hook", h)
        mod.get_axon_ntff_profile_hook = lambda: state["hook"]
        sys.modules["antenv.axon_hooks"] = mod
        import antenv

        antenv.axon_hooks = mod
        from trn_agent_boot.trn_boot import _ntff_profile_via_ctypes

        mod.set_axon_ntff_profile_hook(
            _ntff_profile_via_ctypes("/opt/axon/libaxon_pjrt.so"))
        return mod.get_axon_ntff_profile_hook() is not None
    except Exception:
        sys.modules.pop("antenv.axon_hooks", None)
        return False


def kernel(x, weight, label, _trace=False):
    from concourse.bass_utils import run_bass_kernel_spmd

    if _trace:
        _trace = _ensure_ntff_hook()

    key = "nc"
    if key not in _CACHE:
        _CACHE[key] = _build()
    nc = _CACHE[key]

    in_maps, hcorr, kvec = _prep_inputs(x, weight, label)
    res = run_bass_kernel_spmd(nc, in_maps, core_ids=list(range(N_CORES)),
                               trace=_trace)
    _CACHE["last_result"] = res

    # Gather/unshard: sum the per-core partial sum-exp shards, then finish
    # the (tiny) softmax statistics math on host in f64.
    red = np.zeros((128, N_BT), dtype=np.float64)
    for i in range(N_CORES):
        red += np.asarray(res.results[i]["out"], dtype=np.float64)
    red_b = red.T.reshape(B)            # b = m*128 + p
    sexp_tot = red_b + hcorr
    loss = np.mean(np.log(sexp_tot) + kvec)
    return np.asarray(np.float32(loss))


# revision 33
# speedup vs baseline: 1.0554x; 1.0554x over previous
"""ArcFace loss kernel for 8 Trainium2 NeuronCores.

Strategy (classification/tensor parallel over the class dim C):
  - Host prep: l2-normalize x and weight rows, quantize x_n*16 and
    w_n*16 to fp8-e4m3 and pack the weight shard TRANSPOSED in
    DoubleRow pair layout [ki, (P, s, w)] (d = P*256 + s*128 + ki) so
    each core streams matmul-ready fp8 tiles straight from DRAM.
    Shards are padded with 44 zero classes (to 12544); exp(0)=1 per
    pad column is folded into the host-side correction.
    The ENTIRE label-column (margin) path is computed on host in f64:
    cos_l, phi', the sum-exp correction hcorr_b = e^{S phi'} -
    e^{S cos_l} - PADS, and the per-row affine tail k_b (which folds
    the label-smoothing sum_logits term via wsum = sum_c w_n). Only
    hcorr is shipped to the device ([128, 2*N_BT] f32, with k folded
    into a single host scalar via the final mean).
  - Device per core: cosine*256 = xq @ wq^T via fp8 DoubleRow matmuls
    (K=256/pass, 2 passes) into PSUM [b=128, c<=2048]; ACT Exp
    (scale=S/256) emits bf16 exp tiles (pure exp, no accumulators on
    ACT -- it is the critical engine); DVE tensor_scalar on bf16
    accumulates the per-b sum of exp into per-(m,block) slots.
    A dummy AllReduce at kernel start warms the CC stream / device
    barrier off the critical path. After the last block: fold slots,
    AllReduce the [128, N_BT] sum-exp, then add hcorr, Ln, and a
    ones-matmul mean with the host constant K folded into the final
    scale -- 4 small ops after the collective.

  loss_b = ln(sum_exp_b + hcorr_b) + k_b, averaged over b.
"""

import math
import sys

import numpy as np

S = 30.0
MARGIN = 0.5
EPS = 0.1
COS_M = math.cos(MARGIN)
SIN_M = math.sin(MARGIN)
TH = math.cos(math.pi - MARGIN)
MM = math.sin(math.pi - MARGIN) * MARGIN

B, D, C = 512, 512, 100000
N_CORES = 8
C_SHARD = C // N_CORES            # 12500
C_PAD = 12544                     # +44 zero classes -> 16-aligned widths
PAD_TOTAL = float((C_PAD - C_SHARD) * N_CORES)  # 352 pad columns overall
WIDTHS = [256, 512, 1536, 2048, 2048, 2048, 2048, 2048]  # sum=12544
N_BLK = len(WIDTHS)
# Per-(block, m) engine assignment, interleaved so ACT and DVE stay busy
# concurrently through every block (a block-level split starves one side:
# PSUM is only 2 tiles deep). 2048-blocks run 1-2 Schraudolph tiles on DVE
# + the rest as ACT exp+accum; small blocks 0-2 are ACT exp + DVE accum.
SCH_TILES = {(3, 0), (4, 0), (5, 0), (6, 0), (7, 0)}
ACT_ACC_TILES = {(b, m) for b in range(3, 8) for m in range(4)} - SCH_TILES
# Schraudolph fast-exp: exp(y) ~= bitcast_f32(int32(A*y + B)). The PSUM
# value is 256*cos, so fold S/256 into A. B is the float32 magic constant.
SCH_A = 12102203.1616 * (S / 256.0)
SCH_B = 1064866805.0
SCH_ZERO = float(np.array([1064866805], np.int32).view(np.float32)[0])
N_BT = B // 128                   # 4 b-tiles
ALPHA = 16.0                      # fp8 pre-scale for x_n and w_n
EXP_SCALE = S / (ALPHA * ALPHA)
N_WARM = 8                        # PE p-state warmup matmuls

_REPO = "/opt/trn_rl_repo"
if _REPO not in sys.path:
    sys.path.insert(0, _REPO)

_CACHE = {}


def _build():
    import concourse.bass as bass
    import concourse.mybir as mybir
    from concourse.tile import TileContext

    dt = mybir.dt
    Alu = mybir.AluOpType
    Act = mybir.ActivationFunctionType
    DR = mybir.MatmulPerfMode.DoubleRow

    nc = bass.Bass(num_devices=N_CORES)

    # I/O (per-core shards fed via in_maps)
    xt = nc.declare_dram_parameter("xt", [128, 2 * 1024], dt.uint8,
                                   isOutput=False)  # fp8 x_n^T DR pairs
    wt = nc.declare_dram_parameter("wt", [C_PAD * D], dt.uint8,
                                   isOutput=False)  # fp8 block images, flat
    out = nc.declare_dram_parameter("out", [128, N_BT], dt.float32,
                                    isOutput=True)  # per-core sum-exp

    f32 = dt.float32
    bf16 = dt.bfloat16
    u8 = dt.uint8
    f8 = dt.float8e4
    i32 = dt.int32

    with TileContext(nc) as tc:
        with (
            tc.tile_pool(name="wpool", bufs=4) as wpool,
            tc.tile_pool(name="io", bufs=1) as io,
            tc.tile_pool(name="escr", bufs=4) as escr,
            tc.tile_pool(name="slots", bufs=1) as slots,
            tc.tile_pool(name="stats", bufs=16) as stats,
            tc.tile_pool(name="psum", bufs=2, space="PSUM") as pp,
            tc.tile_pool(name="dram", bufs=4, space="DRAM") as dram,
        ):
            # ---- input staging: first weight block, then x image ----
            xt8 = io.tile([128, 2 * 1024], u8, tag="xt8")
            off = 0
            W0 = WIDTHS[0]
            w_first = wpool.tile([128, 4 * 2048], u8, tag="w")
            img0 = wt[0:128 * 4 * W0].rearrange("(p w) -> p w", p=128)
            nc.sync.dma_start(out=w_first[:, 0:4 * W0], in_=img0[:, :])
            off += 128 * 4 * W0
            nc.sync.dma_start(out=xt8[:, :], in_=xt[:, :])
            xt4 = xt8[:, 0:2048].rearrange("p (P s b) -> p P s b", P=2, s=2)

            sexp_slots = slots.tile([128, N_BT * N_BLK], f32, tag="sexp")

            cc_in = slots.tile([128, N_BT], f32, tag="ccin")

            # ---- PE p-state warmup on a memset tile (overlaps input DMA)
            wrm = stats.tile([128, 1024], bf16, tag="wrm")
            nc.vector.memset(wrm[:, :], 0.0)
            for wi in range(N_WARM):
                ps_w = pp.tile([128, 2048], f32, tag="cos")
                nc.tensor.matmul(ps_w[:, 0:512], lhsT=wrm[:, 0:128],
                                 rhs=wrm[:, 0:512], start=True, stop=True)

            # ---- main streaming loop over class blocks (fp8 DoubleRow) ----
            bi2 = 0
            for W in WIDTHS:
                if bi2 == 0:
                    wtile = w_first
                else:
                    wtile = wpool.tile([128, 4 * 2048], u8, tag="w")
                    img = wt[off:off + 128 * 4 * W].rearrange("(p w) -> p w",
                                                              p=128)
                    if W <= 1536:
                        nc.sync.dma_start(out=wtile[:, 0:4 * W], in_=img[:, :])
                    else:
                        hw = 4 * W // 2
                        nc.sync.dma_start(out=wtile[:, 0:hw], in_=img[:, 0:hw])
                        nc.sync.dma_start(out=wtile[:, hw:4 * W],
                                          in_=img[:, hw:4 * W])
                    off += 128 * 4 * W
                w4 = wtile[:, 0:4 * W].rearrange("p (P s w) -> p P s w",
                                                 P=2, s=2)

                for m in range(N_BT):
                    ps = pp.tile([128, 2048], f32, tag="cos")
                    for reg in range(0, W, 512):
                        RW = min(512, W - reg)
                        for P in range(2):
                            nc.tensor.matmul(
                                ps[:, reg:reg + RW],
                                lhsT=xt4[:, P, :, m * 128:(m + 1) * 128].bitcast(f8),
                                rhs=w4[:, P, :, reg:reg + RW].bitcast(f8),
                                start=(P == 0), stop=(P == 1),
                                perf_mode=DR)
                    acc = sexp_slots[:, m * N_BLK + bi2:m * N_BLK + bi2 + 1]
                    if (bi2, m) in SCH_TILES:
                        # DVE fast-exp: affine into int32 bits, then reduce
                        # the bitcast floats. Offloads ~1/6 of the exp work
                        # from the saturated ACT engine.
                        it = escr.tile([128, 2048], i32, tag="si")
                        nc.vector.tensor_scalar(
                            out=it[:, 0:W], in0=ps[:, 0:W], scalar1=SCH_A,
                            scalar2=SCH_B, op0=Alu.mult, op1=Alu.add)
                        nc.vector.reduce_sum(acc, it[:, 0:W].bitcast(f32),
                                             axis=mybir.AxisListType.X)
                    elif (bi2, m) in ACT_ACC_TILES:
                        # ACT accumulates in-pass (+283ns READ_ACC);
                        # relieves the 1x-rate DVE accumulate path.
                        esc = escr.tile([128, 2048], bf16, tag="et")
                        nc.scalar.activation(esc[:, 0:W], ps[:, 0:W],
                                             Act.Exp, scale=EXP_SCALE,
                                             accum_out=acc)
                    else:
                        esc = escr.tile([128, 2048], bf16, tag="et")
                        nc.scalar.activation(esc[:, 0:W], ps[:, 0:W],
                                             Act.Exp, scale=EXP_SCALE)
                        nc.vector.reduce_sum(acc, esc[:, 0:W],
                                             axis=mybir.AxisListType.X)
                bi2 += 1

            # ---- fold slots -> per-core sum-exp out (host combines) ----
            for m in range(N_BT):
                nc.vector.reduce_sum(
                    cc_in[:, m:m + 1],
                    sexp_slots[:, m * N_BLK:(m + 1) * N_BLK],
                    axis=mybir.AxisListType.X)
            nc.sync.dma_start(out=out[:, :], in_=cc_in[:, :])

    _split_waits(nc, mybir)
    return nc


def _split_waits(nc, mybir):
    """Walrus codegen limits most ISA structs to ONE sync-wait command.

    Tile's scheduler freely attaches several waits to any instruction, so
    hoist surplus waits onto inserted same-engine NoOps (one wait each),
    which execute (and block the engine) immediately before the real
    instruction.
    """
    import bass_rust

    idx = 0
    for blk in nc.main_func.blocks:
        il = blk.instructions
        out = []
        changed = False
        for ins in il:
            si = ins.sync_info
            nwaits = len(si.on_wait) if si is not None else 0
            if si is not None and nwaits > 1:
                waits = list(si.on_wait)
                for w in waits[1:]:
                    noop = mybir.InstNoOp(name=f"I-wsplit-{idx}", ins=[],
                                          outs=[])
                    idx += 1
                    noop.engine = ins.engine
                    noop.sync_info = bass_rust.SyncInfo(on_wait=[w],
                                                        on_update=[])
                    out.append(noop)
                ins.sync_info = bass_rust.SyncInfo(
                    on_wait=waits[:1], on_update=list(si.on_update))
                changed = True
            out.append(ins)
        if changed:
            blk.instructions = out
    return nc


def _to_fp8(a):
    import ml_dtypes
    return np.asarray(a, dtype=ml_dtypes.float8_e4m3)


def _pack_pairs_t(mat_t, width):
    """[512, width] (d-major) -> [128, 4*width] bytes, d = P*256+s*128+ki."""
    return np.ascontiguousarray(
        mat_t.reshape(2, 2, 128, width).transpose(2, 0, 1, 3)
        .reshape(128, 4 * width))


def _prep_inputs(x, weight, label):
    x = np.ascontiguousarray(np.asarray(x, dtype=np.float32))
    weight = np.ascontiguousarray(np.asarray(weight, dtype=np.float32))
    lab = np.asarray(label).astype(np.int64)

    xn = x / np.maximum(np.sqrt((x * x).sum(-1, keepdims=True)), 1e-12)
    wnorm = np.maximum(np.sqrt((weight * weight).sum(-1, keepdims=True)),
                       1e-12)
    wn_all = weight / wnorm

    # ---- host-side label-column (margin) path, all in f64 ----
    wl_n = wn_all[lab].astype(np.float64)                   # [B, D]
    xn64 = xn.astype(np.float64)
    cos_l = np.einsum("bd,bd->b", xn64, wl_n)               # [B]
    sine = np.sqrt(np.clip(1.0 - cos_l * cos_l, 0.0, 1.0))
    phi = cos_l * COS_M - sine * SIN_M
    phip = np.where(cos_l > TH, phi, cos_l - MM)            # phi'
    wsum = wn_all.sum(axis=0, dtype=np.float64)             # [D]
    sumcos = xn64 @ wsum                                    # [B]
    # Pad columns (44 per core) sit in the last block, handled by ACT exp:
    # exp(0)=1 each.
    hcorr = np.exp(S * phip) - np.exp(S * cos_l) - PAD_TOTAL
    kvec = (-(1.0 - EPS) * S * phip
            - (EPS / C) * S * (sumcos - cos_l + phip))      # [B]

    # fp8 x image
    xq = _to_fp8(xn * ALPHA)                                # [B, D]
    xt_pairs = _pack_pairs_t(np.ascontiguousarray(xq.T), B)  # [128, 2048]
    xt_p = np.ascontiguousarray(xt_pairs.view(np.uint8))

    in_maps = []
    for i in range(N_CORES):
        shn = wn_all[i * C_SHARD:(i + 1) * C_SHARD]
        shq = np.zeros((C_PAD, D), dtype=xq.dtype)
        shq[:C_SHARD] = _to_fp8(shn * ALPHA)
        wqT = np.ascontiguousarray(shq.T)        # [D, C_PAD] fp8
        parts = []
        o = 0
        for w in WIDTHS:
            parts.append(_pack_pairs_t(wqT[:, o:o + w], w).reshape(-1))
            o += w
        wt_i = np.ascontiguousarray(
            np.concatenate(parts)).view(np.uint8)  # flat [C_PAD * D]
        in_maps.append({"xt": xt_p, "wt": wt_i})
    return in_maps, hcorr, kvec


def _ensure_ntff_hook():
    """Register the axon NTFF profile hook (missing antenv.axon_hooks shim)."""
    import types

    if "antenv.axon_hooks" in sys.modules:
        return True
    try:
        mod = types.ModuleType("antenv.axon_hooks")
        state = {"hook": None}
        mod.set_axon_ntff_profile_hook = lambda h: state.__setitem__("hook", h)
        mod.get_axon_ntff_profile_hook = lambda: state["hook"]
        sys.modules["antenv.axon_hooks"] = mod
        import antenv

        antenv.axon_hooks = mod
        from trn_agent_boot.trn_boot import _ntff_profile_via_ctypes

        mod.set_axon_ntff_profile_hook(
            _ntff_profile_via_ctypes("/opt/axon/libaxon_pjrt.so"))
        return mod.get_axon_ntff_profile_hook() is not None
    except Exception:
        sys.modules.pop("antenv.axon_hooks", None)
        return False


def kernel(x, weight, label, _trace=False):
    from concourse.bass_utils import run_bass_kernel_spmd

    if _trace:
        _trace = _ensure_ntff_hook()

    key = "nc"
    if key not in _CACHE:
        _CACHE[key] = _build()
    nc = _CACHE[key]

    in_maps, hcorr, kvec = _prep_inputs(x, weight, label)
    res = run_bass_kernel_spmd(nc, in_maps, core_ids=list(range(N_CORES)),
                               trace=_trace)
    _CACHE["last_result"] = res

    # Gather/unshard: sum the per-core partial sum-exp shards, then finish
    # the (tiny) softmax statistics math on host in f64.
    red = np.zeros((128, N_BT), dtype=np.float64)
    for i in range(N_CORES):
        red += np.asarray(res.results[i]["out"], dtype=np.float64)
    red_b = red.T.reshape(B)            # b = m*128 + p
    sexp_tot = red_b + hcorr
    loss = np.mean(np.log(sexp_tot) + kvec)
    return np.asarray(np.float32(loss))
